# revision 14
# baseline (speedup 1.0000x reference)
"""BiMambaBlock Trainium2 kernel (8 NeuronCores, SPMD).

Sharding: core c = br*4 + b*2 + h  (br: 0=fwd/1=bwd branch, b: batch, h: d_inner half).
Each core runs the full LN + mamba pipeline for its (branch, batch) on its
d_inner half (1024 channels), in transposed activation layout [feature, L].
Backward-branch cores receive the input L-reversed from the host, so the
compiled program is identical on all 8 cores.

Cross-core communication:
  - AllReduce of the x_proj partial ([96, L]) between d-half pairs.
  - out_proj+final proj folded into one matmul against host-precomputed
    Q = projW_half @ outW_half; partial outputs combined via a pair
    ReduceScatter (d-half pairs, splitting d_model rows) followed by a
    cross-branch AllGather; the backward block is read L-reversed during the
    final add, which yields the natural-order result on every core.

Host-side work is restricted to weight transforms (transpose/fold/precompute/
dtype cast) and pure data layout (transpose/reverse/slice) of activations.
"""

import numpy as np
import ml_dtypes

import concourse.bass as bass
import concourse.bacc as bacc
import concourse.mybir as mybir
import concourse.tile as tile
from concourse.bass_utils import run_bass_kernel_spmd

F32 = mybir.dt.float32
BF16 = mybir.dt.bfloat16
AF = mybir.ActivationFunctionType
OP = mybir.AluOpType

D = 1024          # d_model
DI = 2048         # d_inner
DL = 1024         # local d_inner (half)
L = 1024
NST = 16          # d_state
B = 2
NC = 8
KT = D // 128     # 8 k-tiles of d_model
TT = DL // 128    # 8 d-tiles of local d_inner
EPS = 1e-5

_CACHED = {}


# ----------------------------------------------------------------------------
# device program
# ----------------------------------------------------------------------------

def build_program():
    nc = bacc.Bacc("TRN2", target_bir_lowering=False, debug=False, num_devices=NC)

    xT_e = nc.declare_dram_parameter("xT", [128, KT, L], BF16, isOutput=False)
    win_e = nc.declare_dram_parameter("WinT", [128, 9, 2 * DL], BF16, isOutput=False)
    convw_e = nc.declare_dram_parameter("convW", [128, TT, 4], F32, isOutput=False)
    convb_e = nc.declare_dram_parameter("convB", [128, TT], F32, isOutput=False)
    dtb_e = nc.declare_dram_parameter("dtb", [128, TT], F32, isOutput=False)
    dsk_e = nc.declare_dram_parameter("Dsk", [128, TT], F32, isOutput=False)
    asc_e = nc.declare_dram_parameter("Ascale", [128, TT, NST], F32, isOutput=False)
    wx_e = nc.declare_dram_parameter("WxT", [128, TT, 96], BF16, isOutput=False)
    wdt_e = nc.declare_dram_parameter("WdtT", [64, DL], BF16, isOutput=False)
    q_e = nc.declare_dram_parameter("Q", [128, TT, D], BF16, isOutput=False)
    xsl_e = nc.declare_dram_parameter("xsl", [128, 4, L], F32, isOutput=False)
    bsl_e = nc.declare_dram_parameter("bsl", [128, 4], F32, isOutput=False)
    id_e = nc.declare_dram_parameter("ident", [128, 128], BF16, isOutput=False)
    out_e = nc.declare_dram_parameter("out_sl", [128, 4, L], F32, isOutput=True)

    with tile.TileContext(nc) as tc:
        with (
            tc.tile_pool(name="persist", bufs=1) as persist,
            tc.tile_pool(name="dram", bufs=1, space="DRAM") as dram,
            tc.tile_pool(name="psA", bufs=2, space="PSUM") as psA,
        ):
            convw = persist.tile([128, TT, 4], F32)
            convb = persist.tile([128, TT], F32)
            dtb = persist.tile([128, TT], F32)
            dsk = persist.tile([128, TT], F32)
            asc = persist.tile([128, TT, NST], F32)
            nc.sync.dma_start(out=convw[:], in_=convw_e[:])
            nc.sync.dma_start(out=convb[:], in_=convb_e[:])
            nc.sync.dma_start(out=dtb[:], in_=dtb_e[:])
            nc.sync.dma_start(out=dsk[:], in_=dsk_e[:])
            nc.sync.dma_start(out=asc[:], in_=asc_e[:])

            xs = persist.tile([128, TT, L], BF16)    # in_proj x-half; later silu(conv(.))
            zs = persist.tile([128, TT, L], BF16)    # silu(z); later y_gated

            # ---------------- phase 0/1: LN + in_proj ----------------
            with (
                tc.tile_pool(name="ph1", bufs=1) as ph1,
                tc.tile_pool(name="ln_ps", bufs=2, space="PSUM") as ln_ps,
            ):
                xsb = ph1.tile([128, KT, L], BF16)
                nc.sync.dma_start(out=xsb[:], in_=xT_e[:])
                win = ph1.tile([128, 9, 2 * DL], BF16)
                nc.sync.dma_start(out=win[:], in_=win_e[:])

                ones = ph1.tile([128, 1], BF16)
                nc.vector.memset(ones[:], 1.0)

                # LN stats over the partition (d_model) axis via PE column sums
                mu = ph1.tile([1, L], F32)
                m2 = ph1.tile([1, L], F32)
                for f in range(0, L, 512):
                    pt = ln_ps.tile([1, 512], F32, tag="lnps")
                    for k in range(KT):
                        nc.tensor.matmul(pt[:], ones[:], xsb[:, k, f:f + 512],
                                         start=(k == 0), stop=(k == KT - 1))
                    nc.vector.tensor_scalar(mu[:, f:f + 512], pt[:], 1.0 / D, None, OP.mult)
                with tc.tile_pool(name="sqp", bufs=3) as sqp:
                    for f in range(0, L, 512):
                        pt = ln_ps.tile([1, 512], F32, tag="lnps")
                        for k in range(KT):
                            sqt = sqp.tile([128, 512], BF16, tag="sqt")
                            nc.scalar.square(sqt[:], xsb[:, k, f:f + 512])
                            nc.tensor.matmul(pt[:], ones[:], sqt[:],
                                             start=(k == 0), stop=(k == KT - 1))
                        nc.vector.tensor_scalar(m2[:, f:f + 512], pt[:], 1.0 / D, None, OP.mult)

                # rs = 1/sqrt(var + eps), Newton-refined after the loose HW sqrt
                musq = ph1.tile([1, L], F32)
                var = ph1.tile([1, L], F32)
                rs = ph1.tile([1, L], F32)
                t0 = ph1.tile([1, L], F32)
                nc.vector.tensor_tensor(musq[:], mu[:], mu[:], OP.mult)
                nc.vector.tensor_tensor(var[:], m2[:], musq[:], OP.subtract)
                nc.vector.tensor_scalar(var[:], var[:], EPS, None, OP.add)
                nc.scalar.sqrt(t0[:], var[:])
                nc.vector.reciprocal(rs[:], t0[:])
                nc.vector.tensor_tensor(t0[:], rs[:], rs[:], OP.mult)
                nc.vector.tensor_tensor(t0[:], var[:], t0[:], OP.mult)
                nc.vector.tensor_scalar(t0[:], t0[:], -0.5, 1.5, OP.mult, OP.add)
                nc.vector.tensor_tensor(rs[:], rs[:], t0[:], OP.mult)
                qrow = ph1.tile([1, L], F32)
                nc.vector.tensor_tensor(qrow[:], mu[:], rs[:], OP.mult)
                nc.vector.tensor_scalar(qrow[:], qrow[:], -1.0, None, OP.mult)

                # broadcast rs to all partitions (bf16)
                ones_row = ph1.tile([1, 128], BF16)
                nc.vector.memset(ones_row[:], 1.0)
                rs16 = ph1.tile([1, L], BF16)
                nc.scalar.copy(rs16[:], rs[:])
                rs_b = ph1.tile([128, L], BF16)
                for f in range(0, L, 512):
                    pt = ln_ps.tile([128, 512], F32, tag="bcps")
                    nc.tensor.matmul(pt[:], ones_row[:], rs16[:, f:f + 512],
                                     start=True, stop=True)
                    nc.scalar.copy(rs_b[:, f:f + 512], pt[:])

                # xhat rows: x * rs (in place); extra k-tile = [q; 1; 0...]
                for k in range(KT):
                    nc.vector.tensor_tensor(xsb[:, k, :], xsb[:, k, :], rs_b[:], OP.mult)
                ex9 = ph1.tile([128, L], BF16)
                nc.vector.memset(ex9[:], 0.0)
                nc.vector.memset(ex9[0:2, :], 1.0)
                nc.scalar.copy(ex9[0:1, :], qrow[:])

                # in_proj: out rows 0..DL-1 -> xs_raw, DL..2DL-1 -> silu(z)
                for m in range(16):
                    for f in range(0, L, 512):
                        pt = psA.tile([128, 512], F32, tag="mm")
                        for k in range(9):
                            rhs = xsb[:, k, f:f + 512] if k < KT else ex9[:, f:f + 512]
                            nc.tensor.matmul(pt[:], win[:, k, m * 128:(m + 1) * 128],
                                             rhs, start=(k == 0), stop=(k == 8))
                        if m < TT:
                            nc.scalar.copy(xs[:, m, f:f + 512], pt[:])
                        else:
                            nc.scalar.activation(zs[:, m - TT, f:f + 512], pt[:], AF.Silu)

            # ---------------- phase 2: conv + silu + x_proj + AR ----------------
            with (
                tc.tile_pool(name="mid", bufs=1) as mid,
                tc.tile_pool(name="xp_ps", bufs=2, space="PSUM") as xp_ps,
                tc.tile_pool(name="cv", bufs=2) as cvp,
            ):
                wx = mid.tile([128, TT, 96], BF16)
                nc.sync.dma_start(out=wx[:], in_=wx_e[:])
                wdt = mid.tile([64, DL], BF16)
                nc.sync.dma_start(out=wdt[:], in_=wdt_e[:])

                # depthwise causal conv along the free dim
                for t in range(TT):
                    cv = cvp.tile([128, L], BF16, tag="cv")
                    nc.vector.tensor_scalar(cv[:], xs[:, t, :], convw[:, t, 3:4], None, OP.mult)
                    for j, k0 in ((2, 1), (1, 2), (0, 3)):
                        nc.vector.scalar_tensor_tensor(
                            cv[:, k0:], xs[:, t, 0:L - k0], convw[:, t, j:j + 1],
                            cv[:, k0:], OP.mult, OP.add)
                    nc.scalar.activation(xs[:, t, :], cv[:], AF.Silu,
                                         bias=convb[:, t:t + 1], scale=1.0)

                # x_proj partial: [96, L]
                xdb = mid.tile([96, L], F32)
                for f in range(0, L, 512):
                    pt = xp_ps.tile([96, 512], F32, tag="xp")
                    for t in range(TT):
                        nc.tensor.matmul(pt[:], wx[:, t, :], xs[:, t, f:f + 512],
                                         start=(t == 0), stop=(t == TT - 1))
                    nc.scalar.copy(xdb[:, f:f + 512], pt[:])

                ar_in = dram.tile([96, L], F32)
                ar_out = dram.tile([96, L], F32)
                nc.sync.dma_start(out=ar_in[:], in_=xdb[:])
                nc.gpsimd.collective_compute(
                    "AllReduce", OP.add,
                    replica_groups=[[0, 1], [2, 3], [4, 5], [6, 7]],
                    ins=[ar_in[:]], outs=[ar_out[:]])
                b_bc = mid.tile([128, NST, L], BF16)
                c_bc = mid.tile([128, NST, L], BF16)
                ident = mid.tile([128, 128], BF16)
                nc.sync.dma_start(out=ident[:], in_=id_e[:])
                dtf_all = mid.tile([128, TT, L], BF16)

                # ---- phase 3a: xdb casts + dt for all tiles (Exp batch, Ln batch) ----
                with tc.tile_pool(name="p3a", bufs=1) as p3a:
                    xdbr = p3a.tile([96, L], F32)
                    nc.sync.dma_start(out=xdbr[:], in_=ar_out[:])
                    dtlow = p3a.tile([64, L], BF16)
                    nc.scalar.copy(dtlow[:], xdbr[0:64, :])
                    bc_bf = p3a.tile([32, L], BF16)
                    nc.scalar.copy(bc_bf[:], xdbr[64:96, :])
                    bc_dram = dram.tile([32, L], BF16)
                    nc.sync.dma_start(out=bc_dram[:], in_=bc_bf[:])

                    for n in range(NST):
                        for dst, base in ((b_bc, 0), (c_bc, NST)):
                            src = bc_dram[base + n: base + n + 1, :]
                            src_b = bass.AP(
                                tensor=src.tensor, offset=src.offset,
                                ap=[[0, 128]] + [list(d) for d in src.ap[1:]])
                            nc.sync.dma_start(out=dst[:, n, :], in_=src_b)

                    et_all = p3a.tile([128, TT, L], BF16)
                    for t in range(TT):
                        for f in range(0, L, 512):
                            pt = psA.tile([128, 512], F32, tag="mm")
                            nc.tensor.matmul(pt[:], wdt[:, t * 128:(t + 1) * 128],
                                             dtlow[:, f:f + 512], start=True, stop=True)
                            nc.scalar.activation(et_all[:, t, f:f + 512], pt[:], AF.Exp,
                                                 bias=dtb[:, t:t + 1], scale=1.0)
                    for t in range(TT):
                        nc.scalar.activation(dtf_all[:, t, :], et_all[:, t, :], AF.Ln,
                                             bias=1.0, scale=1.0)

                # ---------------- phase 3b: per-d-tile selective scan ----------------
                with (
                    tc.tile_pool(name="p3", bufs=1) as p3,
                    tc.tile_pool(name="psY", bufs=2, space="PSUM") as psY,
                ):
                    for t in range(TT):
                        ub = p3.tile([128, L], BF16, tag="ub")
                        nc.vector.tensor_tensor(ub[:], dtf_all[:, t, :], xs[:, t, :], OP.mult)

                        dA = p3.tile([128, NST * L], BF16, tag="dA")
                        dA3 = dA[:].rearrange("p (n l) -> p n l", n=NST)
                        for n in range(NST):
                            nc.scalar.activation(dA3[:, n, :], dtf_all[:, t, :], AF.Exp,
                                                 bias=0.0, scale=asc[:, t, n:n + 1])
                        # isolate the 16 scan segments: zero decay at each head
                        nc.vector.memset(dA3[:, :, 0:1], 0.0)

                        dbx = p3.tile([128, NST * L], BF16, tag="dbx")
                        dbx3 = dbx[:].rearrange("p (n l) -> p n l", n=NST)
                        ub_b = bass.AP(tensor=ub[:].tensor, offset=ub[:].offset,
                                       ap=[list(ub[:].ap[0]), [0, NST]] +
                                          [list(d) for d in ub[:].ap[1:]])
                        nc.vector.tensor_tensor(dbx3[:, :, :], ub_b, b_bc[:, :, :], OP.mult)

                        # selective scan, all 16 state dims in one instruction
                        nc.vector.tensor_tensor_scan(
                            dbx[:], dA[:], dbx[:], 0.0, OP.mult, OP.add)

                        # gh = C_n * h_n in place, then reduce over n on the PE:
                        # accumulate the 16 transposed blocks in PSUM, then
                        # transpose back.
                        nc.vector.tensor_tensor(dbx[:], dbx[:],
                                                c_bc[:].rearrange("p n l -> p (n l)"),
                                                OP.mult)
                        # y = sum_n gh_n: identity-lhsT matmuls accumulate in PSUM
                        yp = psY.tile([128, L], F32, tag="yp")
                        for f in range(0, L, 512):
                            for n in range(NST):
                                nc.tensor.matmul(yp[:, f:f + 512], ident[:],
                                                 dbx3[:, n, f:f + 512],
                                                 start=(n == 0), stop=(n == NST - 1))

                        # gate: yg = (y + xs*D) * silu(z) -> store into zs slot
                        tg = p3.tile([128, L], BF16, tag="tg")
                        nc.vector.scalar_tensor_tensor(
                            tg[:], xs[:, t, :], dsk[:, t:t + 1], yp[:], OP.mult, OP.add)
                        nc.vector.tensor_tensor(zs[:, t, :], tg[:], zs[:, t, :], OP.mult)

            # -------- phase 4: final matmul + RS + AG + add, chunked by dm-half ----
            with tc.tile_pool(name="fin", bufs=1) as fin:
                qw = fin.tile([128, TT, D], BF16)
                nc.sync.dma_start(out=qw[:], in_=q_e[:])
                xsl = fin.tile([128, 4, L], F32)
                nc.sync.dma_start(out=xsl[:], in_=xsl_e[:])
                bsl = fin.tile([128, 4], F32)
                nc.sync.dma_start(out=bsl[:], in_=bsl_e[:])

                with tc.tile_pool(name="fdrain", bufs=4) as fdr:
                    for c in range(2):
                        fin_in = dram.tile([D // 2, L], BF16, tag=f"fin_in{c}")
                        for m in range(c * 4, c * 4 + 4):
                            for f in range(0, L, 512):
                                pt = psA.tile([128, 512], F32, tag="mm")
                                for k in range(TT):
                                    nc.tensor.matmul(
                                        pt[:], qw[:, k, m * 128:(m + 1) * 128],
                                        zs[:, k, f:f + 512],
                                        start=(k == 0), stop=(k == TT - 1))
                                ft = fdr.tile([128, 512], BF16, tag="ft")
                                nc.scalar.copy(ft[:], pt[:])
                                nc.sync.dma_start(
                                    out=fin_in[(m - c * 4) * 128:(m - c * 4 + 1) * 128,
                                               f:f + 512], in_=ft[:])
                        # pair ReduceScatter over d-half pairs: [512, L] -> [256, L]
                        rs_out = dram.tile([D // 4, L], BF16, tag=f"rs_out{c}")
                        nc.gpsimd.collective_compute(
                            "ReduceScatter", OP.add,
                            replica_groups=[[0, 1], [2, 3], [4, 5], [6, 7]],
                            ins=[fin_in[:]], outs=[rs_out[:]])
                        # cross-branch AllGather -> [fwd 256 rows; bwd 256 rows]
                        ag_out = dram.tile([D // 2, L], BF16, tag=f"ag_out{c}")
                        nc.gpsimd.collective_compute(
                            "AllGather", OP.bypass,
                            replica_groups=[[0, 4], [1, 5], [2, 6], [3, 7]],
                            ins=[rs_out[:]], outs=[ag_out[:]])

                        for jj in range(2):
                            j = c * 2 + jj
                            fb = fin.tile([128, L], BF16, tag="fb")
                            bb = fin.tile([128, L], BF16, tag="bb")
                            nc.sync.dma_start(
                                out=fb[:], in_=ag_out[jj * 128:(jj + 1) * 128, :])
                            nc.sync.dma_start(
                                out=bb[:], in_=ag_out[256 + jj * 128: 256 + (jj + 1) * 128, :])
                            # fwd natural + bwd L-reversed -> natural on every core
                            ob = fin.tile([128, L], F32, tag="ob")
                            nc.vector.tensor_tensor(fb[:], fb[:], bb[:, ::-1], OP.add)
                            nc.vector.scalar_tensor_tensor(
                                ob[:], fb[:], bsl[:, j:j + 1], xsl[:, j, :], OP.add, OP.add)
                            nc.sync.dma_start(out=out_e[:, j, :], in_=ob[:])

    nc.finalize()
    return nc


# ----------------------------------------------------------------------------
# host-side input prep
# ----------------------------------------------------------------------------

def _part_layout(a):
    """[T*128, ...] -> [128, T, ...] with row r=t*128+p at [p, t]."""
    T = a.shape[0] // 128
    return np.ascontiguousarray(
        a.reshape(T, 128, *a.shape[1:]).transpose(1, 0, *range(2, a.ndim + 1)))


def make_core_inputs(inputs):
    inp = {k: np.asarray(v, dtype=np.float32) for k, v in inputs.items()}
    ln_w, ln_b = inp["ln_w"], inp["ln_b"]
    bf = ml_dtypes.bfloat16
    in_maps = []
    for c in range(NC):
        br, b, h = c // 4, (c // 2) % 2, c % 2
        p = "f_" if br == 0 else "b_"
        dl = slice(h * DL, (h + 1) * DL)

        xTb = inp["x"][b].T
        if br == 1:
            xTb = xTb[:, ::-1]
        m = {}
        m["xT"] = _part_layout(np.ascontiguousarray(xTb)).astype(bf)

        inW = inp[p + "inW"]
        W_sel = np.concatenate([inW[dl], inW[DI + h * DL: DI + (h + 1) * DL]], 0)
        W_t = W_sel * ln_w[None, :]
        blob = np.zeros((1152, 2 * DL), np.float32)
        blob[0:D] = W_t.T
        blob[D] = W_t.sum(axis=1)          # k1, multiplies the q = -mu*rs row
        blob[D + 1] = W_sel @ ln_b         # k2, multiplies the ones row
        m["WinT"] = _part_layout(blob.reshape(9 * 128, 2 * DL)).astype(bf)

        m["convW"] = _part_layout(inp[p + "convW"][dl, 0, :])
        m["convB"] = _part_layout(inp[p + "convb"][dl])
        m["dtb"] = _part_layout(inp[p + "dtb"][dl])
        m["Dsk"] = _part_layout(inp[p + "Dskip"][dl])
        m["Ascale"] = _part_layout(-np.exp(inp[p + "A_log"][dl]))
        m["WxT"] = _part_layout(
            np.ascontiguousarray(inp[p + "xprojW"][:, dl].T)).astype(bf)
        m["WdtT"] = np.ascontiguousarray(inp[p + "dtW"][dl].T).astype(bf)

        Pm = inp["proj_W"][:, br * 1024:(br + 1) * 1024]
        Om = inp[p + "outW"][:, dl]
        Qm = (Pm.astype(np.float64) @ Om.astype(np.float64)).astype(np.float32)
        m["Q"] = _part_layout(np.ascontiguousarray(Qm.T)).astype(bf)

        xTnat = inp["x"][b].T
        xsl = np.zeros((128, 4, L), np.float32)
        bsl = np.zeros((128, 4), np.float32)
        for j in range(4):
            r0 = (j // 2) * 512 + h * 256 + (j % 2) * 128
            xsl[:, j, :] = xTnat[r0:r0 + 128]
            bsl[:, j] = inp["proj_b"][r0:r0 + 128]
        m["xsl"] = xsl
        m["bsl"] = bsl
        m["ident"] = np.eye(128, dtype=np.float32).astype(ml_dtypes.bfloat16)
        in_maps.append(m)
    return in_maps


def assemble_output(results):
    out = np.zeros((B, L, D), np.float32)
    for b in range(B):
        for h in range(2):
            c = b * 2 + h            # fwd cores hold the canonical result
            sl = results[c]["out_sl"]              # [128, 4, L]
            for j in range(4):
                r0 = (j // 2) * 512 + h * 256 + (j % 2) * 128
                out[b, :, r0:r0 + 128] = sl[:, j, :].T
    return out


def kernel(**inputs):
    if "prog" not in _CACHED:
        _CACHED["prog"] = build_program()
    nc = _CACHED["prog"]
    in_maps = make_core_inputs(inputs)
    r = run_bass_kernel_spmd(nc, in_maps, list(range(NC)))
    return assemble_output(r.results)


# revision 15
# speedup vs baseline: 1.0471x; 1.0471x over previous
"""BiMambaBlock Trainium2 kernel (8 NeuronCores, SPMD).

Sharding: core c = br*4 + b*2 + h  (br: 0=fwd/1=bwd branch, b: batch, h: d_inner half).
Each core runs the full LN + mamba pipeline for its (branch, batch) on its
d_inner half (1024 channels), in transposed activation layout [feature, L].
Backward-branch cores receive the input L-reversed from the host, so the
compiled program is identical on all 8 cores.

Cross-core communication:
  - AllReduce of the x_proj partial ([96, L]) between d-half pairs.
  - out_proj+final proj folded into one matmul against host-precomputed
    Q = projW_half @ outW_half; partial outputs combined via a pair
    ReduceScatter (d-half pairs, splitting d_model rows) followed by a
    cross-branch AllGather; the backward block is read L-reversed during the
    final add, which yields the natural-order result on every core.

Host-side work is restricted to weight transforms (transpose/fold/precompute/
dtype cast) and pure data layout (transpose/reverse/slice) of activations.
"""

import numpy as np
import ml_dtypes

import concourse.bass as bass
import concourse.bacc as bacc
import concourse.mybir as mybir
import concourse.tile as tile
from concourse.bass_utils import run_bass_kernel_spmd

F32 = mybir.dt.float32
BF16 = mybir.dt.bfloat16
AF = mybir.ActivationFunctionType
OP = mybir.AluOpType

D = 1024          # d_model
DI = 2048         # d_inner
DL = 1024         # local d_inner (half)
L = 1024
NST = 16          # d_state
B = 2
NC = 8
KT = D // 128     # 8 k-tiles of d_model
TT = DL // 128    # 8 d-tiles of local d_inner
EPS = 1e-5

_CACHED = {}


# ----------------------------------------------------------------------------
# device program
# ----------------------------------------------------------------------------

def build_program():
    nc = bacc.Bacc("TRN2", target_bir_lowering=False, debug=False, num_devices=NC)

    xT_e = nc.declare_dram_parameter("xT", [128, KT, L], BF16, isOutput=False)
    win_e = nc.declare_dram_parameter("WinT", [128, 9, 2 * DL], BF16, isOutput=False)
    convw_e = nc.declare_dram_parameter("convW", [128, TT, 4], F32, isOutput=False)
    convb_e = nc.declare_dram_parameter("convB", [128, TT], F32, isOutput=False)
    dtb_e = nc.declare_dram_parameter("dtb", [128, TT], F32, isOutput=False)
    dsk_e = nc.declare_dram_parameter("Dsk", [128, TT], F32, isOutput=False)
    asc_e = nc.declare_dram_parameter("Ascale", [128, TT, NST], F32, isOutput=False)
    wx_e = nc.declare_dram_parameter("WxT", [128, TT, 96], BF16, isOutput=False)
    wdt_e = nc.declare_dram_parameter("WdtT", [64, DL], BF16, isOutput=False)
    q_e = nc.declare_dram_parameter("Q", [128, TT, D], BF16, isOutput=False)
    xsl_e = nc.declare_dram_parameter("xsl", [128, 4, L], F32, isOutput=False)
    bsl_e = nc.declare_dram_parameter("bsl", [128, 4], F32, isOutput=False)
    id_e = nc.declare_dram_parameter("ident", [128, 128], BF16, isOutput=False)
    out_e = nc.declare_dram_parameter("out_sl", [128, 4, L], F32, isOutput=True)

    with tile.TileContext(nc) as tc:
        with (
            tc.tile_pool(name="persist", bufs=1) as persist,
            tc.tile_pool(name="dram", bufs=1, space="DRAM") as dram,
            tc.tile_pool(name="psA", bufs=4, space="PSUM") as psA,
        ):
            convw = persist.tile([128, TT, 4], F32)
            convb = persist.tile([128, TT], F32)
            dtb = persist.tile([128, TT], F32)
            dsk = persist.tile([128, TT], F32)
            asc = persist.tile([128, TT, NST], F32)
            nc.sync.dma_start(out=convw[:], in_=convw_e[:])
            nc.sync.dma_start(out=convb[:], in_=convb_e[:])
            nc.sync.dma_start(out=dtb[:], in_=dtb_e[:])
            nc.sync.dma_start(out=dsk[:], in_=dsk_e[:])
            nc.sync.dma_start(out=asc[:], in_=asc_e[:])

            xs = persist.tile([128, TT, L], BF16)    # in_proj x-half; later silu(conv(.))
            zs = persist.tile([128, TT, L], BF16)    # silu(z); later y_gated

            # ---------------- phase 0/1: LN + in_proj ----------------
            with (
                tc.tile_pool(name="ph1", bufs=1) as ph1,
                tc.tile_pool(name="ln_ps", bufs=2, space="PSUM") as ln_ps,
            ):
                xsb = ph1.tile([128, KT, L], BF16)
                nc.sync.dma_start(out=xsb[:], in_=xT_e[:])
                win = ph1.tile([128, 9, 2 * DL], BF16)
                nc.sync.dma_start(out=win[:], in_=win_e[:])

                ones = ph1.tile([128, 1], BF16)
                nc.vector.memset(ones[:], 1.0)

                # LN stats over the partition (d_model) axis via PE column sums
                mu = ph1.tile([1, L], F32)
                m2 = ph1.tile([1, L], F32)
                for f in range(0, L, 512):
                    pt = ln_ps.tile([1, 512], F32, tag="lnps")
                    for k in range(KT):
                        nc.tensor.matmul(pt[:], ones[:], xsb[:, k, f:f + 512],
                                         start=(k == 0), stop=(k == KT - 1))
                    nc.vector.tensor_scalar(mu[:, f:f + 512], pt[:], 1.0 / D, None, OP.mult)
                with tc.tile_pool(name="sqp", bufs=3) as sqp:
                    for f in range(0, L, 512):
                        pt = ln_ps.tile([1, 512], F32, tag="lnps")
                        for k in range(KT):
                            sqt = sqp.tile([128, 512], BF16, tag="sqt")
                            nc.scalar.square(sqt[:], xsb[:, k, f:f + 512])
                            nc.tensor.matmul(pt[:], ones[:], sqt[:],
                                             start=(k == 0), stop=(k == KT - 1))
                        nc.vector.tensor_scalar(m2[:, f:f + 512], pt[:], 1.0 / D, None, OP.mult)

                # rs = 1/sqrt(var + eps), Newton-refined after the loose HW sqrt
                musq = ph1.tile([1, L], F32)
                var = ph1.tile([1, L], F32)
                rs = ph1.tile([1, L], F32)
                t0 = ph1.tile([1, L], F32)
                nc.vector.tensor_tensor(musq[:], mu[:], mu[:], OP.mult)
                nc.vector.tensor_tensor(var[:], m2[:], musq[:], OP.subtract)
                nc.vector.tensor_scalar(var[:], var[:], EPS, None, OP.add)
                nc.scalar.sqrt(t0[:], var[:])
                nc.vector.reciprocal(rs[:], t0[:])
                nc.vector.tensor_tensor(t0[:], rs[:], rs[:], OP.mult)
                nc.vector.tensor_tensor(t0[:], var[:], t0[:], OP.mult)
                nc.vector.tensor_scalar(t0[:], t0[:], -0.5, 1.5, OP.mult, OP.add)
                nc.vector.tensor_tensor(rs[:], rs[:], t0[:], OP.mult)
                qrow = ph1.tile([1, L], F32)
                nc.vector.tensor_tensor(qrow[:], mu[:], rs[:], OP.mult)
                nc.vector.tensor_scalar(qrow[:], qrow[:], -1.0, None, OP.mult)

                # broadcast rs to all partitions (bf16)
                ones_row = ph1.tile([1, 128], BF16)
                nc.vector.memset(ones_row[:], 1.0)
                rs16 = ph1.tile([1, L], BF16)
                nc.scalar.copy(rs16[:], rs[:])
                rs_b = ph1.tile([128, L], BF16)
                for f in range(0, L, 512):
                    pt = ln_ps.tile([128, 512], F32, tag="bcps")
                    nc.tensor.matmul(pt[:], ones_row[:], rs16[:, f:f + 512],
                                     start=True, stop=True)
                    nc.scalar.copy(rs_b[:, f:f + 512], pt[:])

                # xhat rows: x * rs (in place); extra k-tile = [q; 1; 0...]
                for k in range(KT):
                    nc.vector.tensor_tensor(xsb[:, k, :], xsb[:, k, :], rs_b[:], OP.mult)
                ex9 = ph1.tile([128, L], BF16)
                nc.vector.memset(ex9[:], 0.0)
                nc.vector.memset(ex9[0:2, :], 1.0)
                nc.scalar.copy(ex9[0:1, :], qrow[:])

                # in_proj: out rows 0..DL-1 -> xs_raw, DL..2DL-1 -> silu(z)
                for m in range(16):
                    for f in range(0, L, 512):
                        pt = psA.tile([128, 512], F32, tag="mm")
                        for k in range(9):
                            rhs = xsb[:, k, f:f + 512] if k < KT else ex9[:, f:f + 512]
                            nc.tensor.matmul(pt[:], win[:, k, m * 128:(m + 1) * 128],
                                             rhs, start=(k == 0), stop=(k == 8))
                        if m < TT:
                            nc.scalar.copy(xs[:, m, f:f + 512], pt[:])
                        else:
                            nc.scalar.activation(zs[:, m - TT, f:f + 512], pt[:], AF.Silu)

            # ---------------- phase 2: conv + silu + x_proj + AR ----------------
            with (
                tc.tile_pool(name="mid", bufs=1) as mid,
                tc.tile_pool(name="xp_ps", bufs=2, space="PSUM") as xp_ps,
                tc.tile_pool(name="cv", bufs=2) as cvp,
            ):
                wx = mid.tile([128, TT, 96], BF16)
                nc.sync.dma_start(out=wx[:], in_=wx_e[:])
                wdt = mid.tile([64, DL], BF16)
                nc.sync.dma_start(out=wdt[:], in_=wdt_e[:])

                # depthwise causal conv along the free dim
                for t in range(TT):
                    cv = cvp.tile([128, L], BF16, tag="cv")
                    nc.vector.tensor_scalar(cv[:], xs[:, t, :], convw[:, t, 3:4], None, OP.mult)
                    for j, k0 in ((2, 1), (1, 2), (0, 3)):
                        nc.vector.scalar_tensor_tensor(
                            cv[:, k0:], xs[:, t, 0:L - k0], convw[:, t, j:j + 1],
                            cv[:, k0:], OP.mult, OP.add)
                    nc.scalar.activation(xs[:, t, :], cv[:], AF.Silu,
                                         bias=convb[:, t:t + 1], scale=1.0)

                # x_proj partial: [96, L]
                xdb = mid.tile([96, L], F32)
                for f in range(0, L, 512):
                    pt = xp_ps.tile([96, 512], F32, tag="xp")
                    for t in range(TT):
                        nc.tensor.matmul(pt[:], wx[:, t, :], xs[:, t, f:f + 512],
                                         start=(t == 0), stop=(t == TT - 1))
                    nc.scalar.copy(xdb[:, f:f + 512], pt[:])

                ar_in = dram.tile([96, L], F32)
                ar_out = dram.tile([96, L], F32)
                nc.sync.dma_start(out=ar_in[:], in_=xdb[:])
                nc.gpsimd.collective_compute(
                    "AllReduce", OP.add,
                    replica_groups=[[0, 1], [2, 3], [4, 5], [6, 7]],
                    ins=[ar_in[:]], outs=[ar_out[:]])
                b_bc = mid.tile([128, NST, L], BF16)
                c_bc = mid.tile([128, NST, L], BF16)
                ident = mid.tile([128, 128], BF16)
                nc.sync.dma_start(out=ident[:], in_=id_e[:])
                dtf_all = mid.tile([128, TT, L], BF16)

                # ---- phase 3a: xdb casts + dt for all tiles (Exp batch, Ln batch) ----
                with tc.tile_pool(name="p3a", bufs=1) as p3a:
                    xdbr = p3a.tile([96, L], F32)
                    nc.sync.dma_start(out=xdbr[:], in_=ar_out[:])
                    dtlow = p3a.tile([64, L], BF16)
                    nc.scalar.copy(dtlow[:], xdbr[0:64, :])
                    bc_bf = p3a.tile([32, L], BF16)
                    nc.scalar.copy(bc_bf[:], xdbr[64:96, :])
                    bc_dram = dram.tile([32, L], BF16)
                    nc.sync.dma_start(out=bc_dram[:], in_=bc_bf[:])

                    for n in range(NST):
                        for dst, base in ((b_bc, 0), (c_bc, NST)):
                            src = bc_dram[base + n: base + n + 1, :]
                            src_b = bass.AP(
                                tensor=src.tensor, offset=src.offset,
                                ap=[[0, 128]] + [list(d) for d in src.ap[1:]])
                            nc.sync.dma_start(out=dst[:, n, :], in_=src_b)

                    et_all = p3a.tile([128, TT, L], BF16)
                    for t in range(TT):
                        for f in range(0, L, 512):
                            pt = psA.tile([128, 512], F32, tag="mm")
                            nc.tensor.matmul(pt[:], wdt[:, t * 128:(t + 1) * 128],
                                             dtlow[:, f:f + 512], start=True, stop=True)
                            nc.scalar.activation(et_all[:, t, f:f + 512], pt[:], AF.Exp,
                                                 bias=dtb[:, t:t + 1], scale=1.0)
                    for t in range(TT):
                        nc.scalar.activation(dtf_all[:, t, :], et_all[:, t, :], AF.Ln,
                                             bias=1.0, scale=1.0)

                # ---------------- phase 3b: per-d-tile selective scan ----------------
                with (
                    tc.tile_pool(name="p3", bufs=1) as p3,
                    tc.tile_pool(name="psY", bufs=1, space="PSUM") as psY,
                ):
                    for t in range(TT):
                        ub = p3.tile([128, L], BF16, tag="ub")
                        nc.vector.tensor_tensor(ub[:], dtf_all[:, t, :], xs[:, t, :], OP.mult)

                        dA = p3.tile([128, NST * L], BF16, tag="dA")
                        dA3 = dA[:].rearrange("p (n l) -> p n l", n=NST)
                        for n in range(NST):
                            nc.scalar.activation(dA3[:, n, :], dtf_all[:, t, :], AF.Exp,
                                                 bias=0.0, scale=asc[:, t, n:n + 1])
                        # isolate the 16 scan segments: zero decay at each head
                        nc.vector.memset(dA3[:, :, 0:1], 0.0)

                        dbx = p3.tile([128, NST * L], BF16, tag="dbx")
                        dbx3 = dbx[:].rearrange("p (n l) -> p n l", n=NST)
                        ub_b = bass.AP(tensor=ub[:].tensor, offset=ub[:].offset,
                                       ap=[list(ub[:].ap[0]), [0, NST]] +
                                          [list(d) for d in ub[:].ap[1:]])
                        nc.vector.tensor_tensor(dbx3[:, :, :], ub_b, b_bc[:, :, :], OP.mult)

                        # selective scan, all 16 state dims in one instruction
                        nc.vector.tensor_tensor_scan(
                            dbx[:], dA[:], dbx[:], 0.0, OP.mult, OP.add)

                        # gh = C_n * h_n in place, then reduce over n on the PE:
                        # accumulate the 16 transposed blocks in PSUM, then
                        # transpose back.
                        for n in range(NST):
                            nc.vector.tensor_tensor(dbx3[:, n, :], dbx3[:, n, :],
                                                    c_bc[:, n, :], OP.mult)
                        # y = sum_n gh_n: identity-lhsT matmuls accumulate in PSUM
                        yp = psY.tile([128, L], F32, tag="yp")
                        for f in range(0, L, 512):
                            for n in range(NST):
                                nc.tensor.matmul(yp[:, f:f + 512], ident[:],
                                                 dbx3[:, n, f:f + 512],
                                                 start=(n == 0), stop=(n == NST - 1))

                        # gate: yg = (y + xs*D) * silu(z) -> store into zs slot
                        tg = p3.tile([128, L], BF16, tag="tg")
                        nc.vector.scalar_tensor_tensor(
                            tg[:], xs[:, t, :], dsk[:, t:t + 1], yp[:], OP.mult, OP.add)
                        nc.vector.tensor_tensor(zs[:, t, :], tg[:], zs[:, t, :], OP.mult)

            # -------- phase 4: final matmul + RS + AG + add, chunked by dm-half ----
            with tc.tile_pool(name="fin", bufs=1) as fin:
                qw = fin.tile([128, TT, D], BF16)
                nc.sync.dma_start(out=qw[:], in_=q_e[:])
                xsl = fin.tile([128, 4, L], F32)
                nc.sync.dma_start(out=xsl[:], in_=xsl_e[:])
                bsl = fin.tile([128, 4], F32)
                nc.sync.dma_start(out=bsl[:], in_=bsl_e[:])

                with tc.tile_pool(name="fdrain", bufs=4) as fdr:
                    for c in range(2):
                        fin_in = dram.tile([D // 2, L], BF16, tag=f"fin_in{c}")
                        for m in range(c * 4, c * 4 + 4):
                            for f in range(0, L, 512):
                                pt = psA.tile([128, 512], F32, tag="mm")
                                for k in range(TT):
                                    nc.tensor.matmul(
                                        pt[:], qw[:, k, m * 128:(m + 1) * 128],
                                        zs[:, k, f:f + 512],
                                        start=(k == 0), stop=(k == TT - 1))
                                ft = fdr.tile([128, 512], BF16, tag="ft")
                                nc.scalar.copy(ft[:], pt[:])
                                nc.sync.dma_start(
                                    out=fin_in[(m - c * 4) * 128:(m - c * 4 + 1) * 128,
                                               f:f + 512], in_=ft[:])
                        # pair ReduceScatter over d-half pairs: [512, L] -> [256, L]
                        rs_out = dram.tile([D // 4, L], BF16, tag=f"rs_out{c}")
                        nc.gpsimd.collective_compute(
                            "ReduceScatter", OP.add,
                            replica_groups=[[0, 1], [2, 3], [4, 5], [6, 7]],
                            ins=[fin_in[:]], outs=[rs_out[:]])
                        # cross-branch AllGather -> [fwd 256 rows; bwd 256 rows]
                        ag_out = dram.tile([D // 2, L], BF16, tag=f"ag_out{c}")
                        nc.gpsimd.collective_compute(
                            "AllGather", OP.bypass,
                            replica_groups=[[0, 4], [1, 5], [2, 6], [3, 7]],
                            ins=[rs_out[:]], outs=[ag_out[:]])

                        for jj in range(2):
                            j = c * 2 + jj
                            fb = fin.tile([128, L], BF16, tag="fb")
                            bb = fin.tile([128, L], BF16, tag="bb")
                            nc.sync.dma_start(
                                out=fb[:], in_=ag_out[jj * 128:(jj + 1) * 128, :])
                            nc.sync.dma_start(
                                out=bb[:], in_=ag_out[256 + jj * 128: 256 + (jj + 1) * 128, :])
                            # fwd natural + bwd L-reversed -> natural on every core
                            ob = fin.tile([128, L], F32, tag="ob")
                            nc.vector.tensor_tensor(fb[:], fb[:], bb[:, ::-1], OP.add)
                            nc.vector.scalar_tensor_tensor(
                                ob[:], fb[:], bsl[:, j:j + 1], xsl[:, j, :], OP.add, OP.add)
                            nc.sync.dma_start(out=out_e[:, j, :], in_=ob[:])

    nc.finalize()
    return nc


# ----------------------------------------------------------------------------
# host-side input prep
# ----------------------------------------------------------------------------

def _part_layout(a):
    """[T*128, ...] -> [128, T, ...] with row r=t*128+p at [p, t]."""
    T = a.shape[0] // 128
    return np.ascontiguousarray(
        a.reshape(T, 128, *a.shape[1:]).transpose(1, 0, *range(2, a.ndim + 1)))


def make_core_inputs(inputs):
    inp = {k: np.asarray(v, dtype=np.float32) for k, v in inputs.items()}
    ln_w, ln_b = inp["ln_w"], inp["ln_b"]
    bf = ml_dtypes.bfloat16
    in_maps = []
    for c in range(NC):
        br, b, h = c // 4, (c // 2) % 2, c % 2
        p = "f_" if br == 0 else "b_"
        dl = slice(h * DL, (h + 1) * DL)

        xTb = inp["x"][b].T
        if br == 1:
            xTb = xTb[:, ::-1]
        m = {}
        m["xT"] = _part_layout(np.ascontiguousarray(xTb)).astype(bf)

        inW = inp[p + "inW"]
        W_sel = np.concatenate([inW[dl], inW[DI + h * DL: DI + (h + 1) * DL]], 0)
        W_t = W_sel * ln_w[None, :]
        blob = np.zeros((1152, 2 * DL), np.float32)
        blob[0:D] = W_t.T
        blob[D] = W_t.sum(axis=1)          # k1, multiplies the q = -mu*rs row
        blob[D + 1] = W_sel @ ln_b         # k2, multiplies the ones row
        m["WinT"] = _part_layout(blob.reshape(9 * 128, 2 * DL)).astype(bf)

        m["convW"] = _part_layout(inp[p + "convW"][dl, 0, :])
        m["convB"] = _part_layout(inp[p + "convb"][dl])
        m["dtb"] = _part_layout(inp[p + "dtb"][dl])
        m["Dsk"] = _part_layout(inp[p + "Dskip"][dl])
        m["Ascale"] = _part_layout(-np.exp(inp[p + "A_log"][dl]))
        m["WxT"] = _part_layout(
            np.ascontiguousarray(inp[p + "xprojW"][:, dl].T)).astype(bf)
        m["WdtT"] = np.ascontiguousarray(inp[p + "dtW"][dl].T).astype(bf)

        Pm = inp["proj_W"][:, br * 1024:(br + 1) * 1024]
        Om = inp[p + "outW"][:, dl]
        Qm = (Pm.astype(np.float64) @ Om.astype(np.float64)).astype(np.float32)
        m["Q"] = _part_layout(np.ascontiguousarray(Qm.T)).astype(bf)

        xTnat = inp["x"][b].T
        xsl = np.zeros((128, 4, L), np.float32)
        bsl = np.zeros((128, 4), np.float32)
        for j in range(4):
            r0 = (j // 2) * 512 + h * 256 + (j % 2) * 128
            xsl[:, j, :] = xTnat[r0:r0 + 128]
            bsl[:, j] = inp["proj_b"][r0:r0 + 128]
        m["xsl"] = xsl
        m["bsl"] = bsl
        m["ident"] = np.eye(128, dtype=np.float32).astype(ml_dtypes.bfloat16)
        in_maps.append(m)
    return in_maps


def assemble_output(results):
    out = np.zeros((B, L, D), np.float32)
    for b in range(B):
        for h in range(2):
            c = b * 2 + h            # fwd cores hold the canonical result
            sl = results[c]["out_sl"]              # [128, 4, L]
            for j in range(4):
                r0 = (j // 2) * 512 + h * 256 + (j % 2) * 128
                out[b, :, r0:r0 + 128] = sl[:, j, :].T
    return out


def kernel(**inputs):
    if "prog" not in _CACHED:
        _CACHED["prog"] = build_program()
    nc = _CACHED["prog"]
    in_maps = make_core_inputs(inputs)
    r = run_bass_kernel_spmd(nc, in_maps, list(range(NC)))
    return assemble_output(r.results)


# revision 16
# speedup vs baseline: 1.0583x; 1.0107x over previous
"""BiMambaBlock Trainium2 kernel (8 NeuronCores, SPMD).

Sharding: core c = br*4 + b*2 + h  (br: 0=fwd/1=bwd branch, b: batch, h: d_inner half).
Each core runs the full LN + mamba pipeline for its (branch, batch) on its
d_inner half (1024 channels), in transposed activation layout [feature, L].
Backward-branch cores receive the input L-reversed from the host, so the
compiled program is identical on all 8 cores.

Cross-core communication:
  - AllReduce of the x_proj partial ([96, L]) between d-half pairs.
  - out_proj+final proj folded into one matmul against host-precomputed
    Q = projW_half @ outW_half; partial outputs combined via a pair
    ReduceScatter (d-half pairs, splitting d_model rows) followed by a
    cross-branch AllGather; the backward block is read L-reversed during the
    final add, which yields the natural-order result on every core.

Host-side work is restricted to weight transforms (transpose/fold/precompute/
dtype cast) and pure data layout (transpose/reverse/slice) of activations.
"""

import numpy as np
import ml_dtypes

import concourse.bass as bass
import concourse.bacc as bacc
import concourse.mybir as mybir
import concourse.tile as tile
from concourse.bass_utils import run_bass_kernel_spmd

F32 = mybir.dt.float32
BF16 = mybir.dt.bfloat16
AF = mybir.ActivationFunctionType
OP = mybir.AluOpType

D = 1024          # d_model
DI = 2048         # d_inner
DL = 1024         # local d_inner (half)
L = 1024
NST = 16          # d_state
B = 2
NC = 8
KT = D // 128     # 8 k-tiles of d_model
TT = DL // 128    # 8 d-tiles of local d_inner
EPS = 1e-5

_CACHED = {}


# ----------------------------------------------------------------------------
# device program
# ----------------------------------------------------------------------------

def build_program():
    nc = bacc.Bacc("TRN2", target_bir_lowering=False, debug=False, num_devices=NC)

    xT_e = nc.declare_dram_parameter("xT", [128, KT, L], BF16, isOutput=False)
    win_e = nc.declare_dram_parameter("WinT", [128, 9, 2 * DL], BF16, isOutput=False)
    convw_e = nc.declare_dram_parameter("convW", [128, TT, 4], F32, isOutput=False)
    convb_e = nc.declare_dram_parameter("convB", [128, TT], F32, isOutput=False)
    dtb_e = nc.declare_dram_parameter("dtb", [128, TT], F32, isOutput=False)
    dsk_e = nc.declare_dram_parameter("Dsk", [128, TT], F32, isOutput=False)
    asc_e = nc.declare_dram_parameter("Ascale", [128, TT, NST], F32, isOutput=False)
    wx_e = nc.declare_dram_parameter("WxT", [128, TT, 96], BF16, isOutput=False)
    wdt_e = nc.declare_dram_parameter("WdtT", [64, DL], BF16, isOutput=False)
    q_e = nc.declare_dram_parameter("Q", [128, TT, D], BF16, isOutput=False)
    xsl_e = nc.declare_dram_parameter("xsl", [128, 4, L], F32, isOutput=False)
    bsl_e = nc.declare_dram_parameter("bsl", [128, 4], F32, isOutput=False)
    id_e = nc.declare_dram_parameter("ident", [128, 128], BF16, isOutput=False)
    out_e = nc.declare_dram_parameter("out_sl", [128, 4, L], F32, isOutput=True)

    with tile.TileContext(nc) as tc:
        with (
            tc.tile_pool(name="persist", bufs=1) as persist,
            tc.tile_pool(name="dram", bufs=1, space="DRAM") as dram,
            tc.tile_pool(name="psA", bufs=4, space="PSUM") as psA,
        ):
            convw = persist.tile([128, TT, 4], F32)
            convb = persist.tile([128, TT], F32)
            dtb = persist.tile([128, TT], F32)
            dsk = persist.tile([128, TT], F32)
            asc = persist.tile([128, TT, NST], F32)
            nc.sync.dma_start(out=convw[:], in_=convw_e[:])
            nc.sync.dma_start(out=convb[:], in_=convb_e[:])
            nc.sync.dma_start(out=dtb[:], in_=dtb_e[:])
            nc.sync.dma_start(out=dsk[:], in_=dsk_e[:])
            nc.sync.dma_start(out=asc[:], in_=asc_e[:])

            xs = persist.tile([128, TT, L], BF16)    # in_proj x-half; later silu(conv(.))
            zs = persist.tile([128, TT, L], BF16)    # silu(z); later y_gated

            # ---------------- phase 0/1: LN + in_proj ----------------
            with (
                tc.tile_pool(name="ph1", bufs=1) as ph1,
                tc.tile_pool(name="ln_ps", bufs=2, space="PSUM") as ln_ps,
            ):
                xsb = ph1.tile([128, KT, L], BF16)
                nc.sync.dma_start(out=xsb[:], in_=xT_e[:])
                win = ph1.tile([128, 9, 2 * DL], BF16)
                nc.sync.dma_start(out=win[:], in_=win_e[:])

                ones = ph1.tile([128, 1], BF16)
                nc.vector.memset(ones[:], 1.0)

                # LN stats over the partition (d_model) axis via PE column sums
                mu = ph1.tile([1, L], F32)
                m2 = ph1.tile([1, L], F32)
                for f in range(0, L, 512):
                    pt = ln_ps.tile([1, 512], F32, tag="lnps")
                    for k in range(KT):
                        nc.tensor.matmul(pt[:], ones[:], xsb[:, k, f:f + 512],
                                         start=(k == 0), stop=(k == KT - 1))
                    nc.vector.tensor_scalar(mu[:, f:f + 512], pt[:], 1.0 / D, None, OP.mult)
                with tc.tile_pool(name="sqp", bufs=3) as sqp:
                    for f in range(0, L, 512):
                        pt = ln_ps.tile([1, 512], F32, tag="lnps")
                        for k in range(KT):
                            sqt = sqp.tile([128, 512], BF16, tag="sqt")
                            nc.scalar.square(sqt[:], xsb[:, k, f:f + 512])
                            nc.tensor.matmul(pt[:], ones[:], sqt[:],
                                             start=(k == 0), stop=(k == KT - 1))
                        nc.vector.tensor_scalar(m2[:, f:f + 512], pt[:], 1.0 / D, None, OP.mult)

                # rs = 1/sqrt(var + eps), Newton-refined after the loose HW sqrt
                musq = ph1.tile([1, L], F32)
                var = ph1.tile([1, L], F32)
                rs = ph1.tile([1, L], F32)
                t0 = ph1.tile([1, L], F32)
                nc.vector.tensor_tensor(musq[:], mu[:], mu[:], OP.mult)
                nc.vector.tensor_tensor(var[:], m2[:], musq[:], OP.subtract)
                nc.vector.tensor_scalar(var[:], var[:], EPS, None, OP.add)
                nc.scalar.sqrt(t0[:], var[:])
                nc.vector.reciprocal(rs[:], t0[:])
                nc.vector.tensor_tensor(t0[:], rs[:], rs[:], OP.mult)
                nc.vector.tensor_tensor(t0[:], var[:], t0[:], OP.mult)
                nc.vector.tensor_scalar(t0[:], t0[:], -0.5, 1.5, OP.mult, OP.add)
                nc.vector.tensor_tensor(rs[:], rs[:], t0[:], OP.mult)
                qrow = ph1.tile([1, L], F32)
                nc.vector.tensor_tensor(qrow[:], mu[:], rs[:], OP.mult)
                nc.vector.tensor_scalar(qrow[:], qrow[:], -1.0, None, OP.mult)

                # broadcast rs to all partitions (bf16)
                ones_row = ph1.tile([1, 128], BF16)
                nc.vector.memset(ones_row[:], 1.0)
                rs16 = ph1.tile([1, L], BF16)
                nc.scalar.copy(rs16[:], rs[:])
                rs_b = ph1.tile([128, L], BF16)
                for f in range(0, L, 512):
                    pt = ln_ps.tile([128, 512], F32, tag="bcps")
                    nc.tensor.matmul(pt[:], ones_row[:], rs16[:, f:f + 512],
                                     start=True, stop=True)
                    nc.scalar.copy(rs_b[:, f:f + 512], pt[:])

                # xhat rows: x * rs (in place); extra k-tile = [q; 1; 0...]
                for k in range(KT):
                    nc.vector.tensor_tensor(xsb[:, k, :], xsb[:, k, :], rs_b[:], OP.mult)
                ex9 = ph1.tile([128, L], BF16)
                nc.vector.memset(ex9[:], 0.0)
                nc.vector.memset(ex9[0:2, :], 1.0)
                nc.scalar.copy(ex9[0:1, :], qrow[:])

                # in_proj: out rows 0..DL-1 -> xs_raw, DL..2DL-1 -> silu(z)
                for m in range(16):
                    for f in range(0, L, 512):
                        pt = psA.tile([128, 512], F32, tag="mm")
                        for k in range(9):
                            rhs = xsb[:, k, f:f + 512] if k < KT else ex9[:, f:f + 512]
                            nc.tensor.matmul(pt[:], win[:, k, m * 128:(m + 1) * 128],
                                             rhs, start=(k == 0), stop=(k == 8))
                        if m < TT:
                            nc.scalar.copy(xs[:, m, f:f + 512], pt[:])
                        else:
                            nc.scalar.activation(zs[:, m - TT, f:f + 512], pt[:], AF.Silu)

            # ---------------- phase 2: conv + silu + x_proj + AR ----------------
            with (
                tc.tile_pool(name="mid", bufs=1) as mid,
                tc.tile_pool(name="xp_ps", bufs=2, space="PSUM") as xp_ps,
                tc.tile_pool(name="cv", bufs=2) as cvp,
            ):
                wx = mid.tile([128, TT, 96], BF16)
                nc.sync.dma_start(out=wx[:], in_=wx_e[:])
                wdt = mid.tile([64, DL], BF16)
                nc.sync.dma_start(out=wdt[:], in_=wdt_e[:])

                # depthwise causal conv along the free dim
                for t in range(TT):
                    cv = cvp.tile([128, L], BF16, tag="cv")
                    nc.vector.tensor_scalar(cv[:], xs[:, t, :], convw[:, t, 3:4], None, OP.mult)
                    for j, k0 in ((2, 1), (1, 2), (0, 3)):
                        nc.vector.scalar_tensor_tensor(
                            cv[:, k0:], xs[:, t, 0:L - k0], convw[:, t, j:j + 1],
                            cv[:, k0:], OP.mult, OP.add)
                    nc.scalar.activation(xs[:, t, :], cv[:], AF.Silu,
                                         bias=convb[:, t:t + 1], scale=1.0)

                # x_proj partial: [96, L]
                xdb = mid.tile([96, L], F32)
                for f in range(0, L, 512):
                    pt = xp_ps.tile([96, 512], F32, tag="xp")
                    for t in range(TT):
                        nc.tensor.matmul(pt[:], wx[:, t, :], xs[:, t, f:f + 512],
                                         start=(t == 0), stop=(t == TT - 1))
                    nc.scalar.copy(xdb[:, f:f + 512], pt[:])

                ar_in = dram.tile([96, L], F32)
                ar_out = dram.tile([96, L], F32)
                nc.sync.dma_start(out=ar_in[:], in_=xdb[:])
                nc.gpsimd.collective_compute(
                    "AllReduce", OP.add,
                    replica_groups=[[0, 1], [2, 3], [4, 5], [6, 7]],
                    ins=[ar_in[:]], outs=[ar_out[:]])
                b_bc = mid.tile([128, NST, L], BF16)
                c_bc = mid.tile([128, NST, L], BF16)
                ident = mid.tile([128, 128], BF16)
                nc.sync.dma_start(out=ident[:], in_=id_e[:])
                dtf_all = mid.tile([128, TT, L], BF16)

                # ---- phase 3a: xdb casts + dt for all tiles (Exp batch, Ln batch) ----
                with tc.tile_pool(name="p3a", bufs=1) as p3a:
                    xdbr = p3a.tile([96, L], F32)
                    nc.sync.dma_start(out=xdbr[:], in_=ar_out[:])
                    dtlow = p3a.tile([64, L], BF16)
                    nc.scalar.copy(dtlow[:], xdbr[0:64, :])
                    bc_bf = p3a.tile([32, L], BF16)
                    nc.scalar.copy(bc_bf[:], xdbr[64:96, :])
                    bc_dram = dram.tile([32, L], BF16)
                    nc.sync.dma_start(out=bc_dram[:], in_=bc_bf[:])

                    for dst, base in ((b_bc, 0), (c_bc, NST)):
                        for n in range(NST):
                            src = bc_dram[base + n: base + n + 1, :]
                            src_b = bass.AP(
                                tensor=src.tensor, offset=src.offset,
                                ap=[[0, 128]] + [list(d) for d in src.ap[1:]])
                            nc.sync.dma_start(out=dst[:, n, :], in_=src_b)

                    et_all = p3a.tile([128, TT, L], BF16)
                    for t in range(TT):
                        for f in range(0, L, 512):
                            pt = psA.tile([128, 512], F32, tag="mm")
                            nc.tensor.matmul(pt[:], wdt[:, t * 128:(t + 1) * 128],
                                             dtlow[:, f:f + 512], start=True, stop=True)
                            nc.scalar.activation(et_all[:, t, f:f + 512], pt[:], AF.Exp,
                                                 bias=dtb[:, t:t + 1], scale=1.0)
                    for t in range(TT):
                        nc.scalar.activation(dtf_all[:, t, :], et_all[:, t, :], AF.Ln,
                                             bias=1.0, scale=1.0)

                # ---------------- phase 3b: per-d-tile selective scan ----------------
                with (
                    tc.tile_pool(name="p3", bufs=1) as p3,
                    tc.tile_pool(name="psY", bufs=1, space="PSUM") as psY,
                ):
                    for t in range(TT):
                        ub = p3.tile([128, L], BF16, tag="ub")
                        nc.vector.tensor_tensor(ub[:], dtf_all[:, t, :], xs[:, t, :], OP.mult)

                        dA = p3.tile([128, NST * L], BF16, tag="dA")
                        dA3 = dA[:].rearrange("p (n l) -> p n l", n=NST)
                        for n in range(NST):
                            nc.scalar.activation(dA3[:, n, :], dtf_all[:, t, :], AF.Exp,
                                                 bias=0.0, scale=asc[:, t, n:n + 1])
                        # isolate the 16 scan segments: zero decay at each head
                        nc.vector.memset(dA3[:, :, 0:1], 0.0)

                        dbx = p3.tile([128, NST * L], BF16, tag="dbx")
                        dbx3 = dbx[:].rearrange("p (n l) -> p n l", n=NST)
                        ub_b = bass.AP(tensor=ub[:].tensor, offset=ub[:].offset,
                                       ap=[list(ub[:].ap[0]), [0, NST]] +
                                          [list(d) for d in ub[:].ap[1:]])
                        nc.vector.tensor_tensor(dbx3[:, :, :], ub_b, b_bc[:, :, :], OP.mult)

                        # selective scan, all 16 state dims in one instruction
                        nc.vector.tensor_tensor_scan(
                            dbx[:], dA[:], dbx[:], 0.0, OP.mult, OP.add)

                        # gh = C_n * h_n in place, then reduce over n on the PE:
                        # accumulate the 16 transposed blocks in PSUM, then
                        # transpose back.
                        for g in range(0, NST, 4):
                            nc.vector.tensor_tensor(
                                dbx[:, g * L:(g + 4) * L], dbx[:, g * L:(g + 4) * L],
                                c_bc[:, g, :].rearrange("p l -> p l") if False else
                                bass.AP(tensor=c_bc[:].tensor, offset=c_bc[:, g, :].offset,
                                        ap=[list(c_bc[:].ap[0]), [L, 4], [1, L]]),
                                OP.mult)
                        # y = sum_n gh_n: identity-lhsT matmuls accumulate in PSUM
                        yp = psY.tile([128, L], F32, tag="yp")
                        for f in range(0, L, 512):
                            for n in range(NST):
                                nc.tensor.matmul(yp[:, f:f + 512], ident[:],
                                                 dbx3[:, n, f:f + 512],
                                                 start=(n == 0), stop=(n == NST - 1))

                        # gate: yg = (y + xs*D) * silu(z) -> store into zs slot
                        tg = p3.tile([128, L], BF16, tag="tg")
                        nc.vector.scalar_tensor_tensor(
                            tg[:], xs[:, t, :], dsk[:, t:t + 1], yp[:], OP.mult, OP.add)
                        nc.vector.tensor_tensor(zs[:, t, :], tg[:], zs[:, t, :], OP.mult)

            # -------- phase 4: final matmul + RS + AG + add, chunked by dm-half ----
            with tc.tile_pool(name="fin", bufs=1) as fin:
                qw = fin.tile([128, TT, D], BF16)
                nc.sync.dma_start(out=qw[:], in_=q_e[:])
                xsl = fin.tile([128, 4, L], F32)
                nc.sync.dma_start(out=xsl[:], in_=xsl_e[:])
                bsl = fin.tile([128, 4], F32)
                nc.sync.dma_start(out=bsl[:], in_=bsl_e[:])

                with tc.tile_pool(name="fdrain", bufs=4) as fdr:
                    fin_in = dram.tile([D, L], BF16)
                    for m in range(KT):
                        for f in range(0, L, 512):
                            pt = psA.tile([128, 512], F32, tag="mm")
                            for k in range(TT):
                                nc.tensor.matmul(
                                    pt[:], qw[:, k, m * 128:(m + 1) * 128],
                                    zs[:, k, f:f + 512],
                                    start=(k == 0), stop=(k == TT - 1))
                            ft = fdr.tile([128, 512], BF16, tag="ft")
                            nc.scalar.copy(ft[:], pt[:])
                            nc.sync.dma_start(
                                out=fin_in[m * 128:(m + 1) * 128, f:f + 512], in_=ft[:])

                    # pair ReduceScatter over d-half pairs: [1024, L] -> [512, L]
                    rs_out = dram.tile([D // 2, L], BF16)
                    nc.gpsimd.collective_compute(
                        "ReduceScatter", OP.add,
                        replica_groups=[[0, 1], [2, 3], [4, 5], [6, 7]],
                        ins=[fin_in[:]], outs=[rs_out[:]])
                    # cross-branch AllGather -> [fwd 512 rows; bwd 512 rows]
                    ag_out = dram.tile([D, L], BF16)
                    nc.gpsimd.collective_compute(
                        "AllGather", OP.bypass,
                        replica_groups=[[0, 4], [1, 5], [2, 6], [3, 7]],
                        ins=[rs_out[:]], outs=[ag_out[:]])

                    for j in range(4):
                        fb = fin.tile([128, L], BF16, tag="fb")
                        bb = fin.tile([128, L], BF16, tag="bb")
                        nc.sync.dma_start(out=fb[:], in_=ag_out[j * 128:(j + 1) * 128, :])
                        nc.sync.dma_start(
                            out=bb[:], in_=ag_out[512 + j * 128: 512 + (j + 1) * 128, :])
                        # fwd natural + bwd L-reversed -> natural on every core
                        ob = fin.tile([128, L], F32, tag="ob")
                        nc.vector.tensor_tensor(fb[:], fb[:], bb[:, ::-1], OP.add)
                        nc.vector.scalar_tensor_tensor(
                            ob[:], fb[:], bsl[:, j:j + 1], xsl[:, j, :], OP.add, OP.add)
                        nc.sync.dma_start(out=out_e[:, j, :], in_=ob[:])

    nc.finalize()
    return nc


# ----------------------------------------------------------------------------
# host-side input prep
# ----------------------------------------------------------------------------

def _part_layout(a):
    """[T*128, ...] -> [128, T, ...] with row r=t*128+p at [p, t]."""
    T = a.shape[0] // 128
    return np.ascontiguousarray(
        a.reshape(T, 128, *a.shape[1:]).transpose(1, 0, *range(2, a.ndim + 1)))


def make_core_inputs(inputs):
    inp = {k: np.asarray(v, dtype=np.float32) for k, v in inputs.items()}
    ln_w, ln_b = inp["ln_w"], inp["ln_b"]
    bf = ml_dtypes.bfloat16
    in_maps = []
    for c in range(NC):
        br, b, h = c // 4, (c // 2) % 2, c % 2
        p = "f_" if br == 0 else "b_"
        dl = slice(h * DL, (h + 1) * DL)

        xTb = inp["x"][b].T
        if br == 1:
            xTb = xTb[:, ::-1]
        m = {}
        m["xT"] = _part_layout(np.ascontiguousarray(xTb)).astype(bf)

        inW = inp[p + "inW"]
        W_sel = np.concatenate([inW[dl], inW[DI + h * DL: DI + (h + 1) * DL]], 0)
        W_t = W_sel * ln_w[None, :]
        blob = np.zeros((1152, 2 * DL), np.float32)
        blob[0:D] = W_t.T
        blob[D] = W_t.sum(axis=1)          # k1, multiplies the q = -mu*rs row
        blob[D + 1] = W_sel @ ln_b         # k2, multiplies the ones row
        m["WinT"] = _part_layout(blob.reshape(9 * 128, 2 * DL)).astype(bf)

        m["convW"] = _part_layout(inp[p + "convW"][dl, 0, :])
        m["convB"] = _part_layout(inp[p + "convb"][dl])
        m["dtb"] = _part_layout(inp[p + "dtb"][dl])
        m["Dsk"] = _part_layout(inp[p + "Dskip"][dl])
        m["Ascale"] = _part_layout(-np.exp(inp[p + "A_log"][dl]))
        m["WxT"] = _part_layout(
            np.ascontiguousarray(inp[p + "xprojW"][:, dl].T)).astype(bf)
        m["WdtT"] = np.ascontiguousarray(inp[p + "dtW"][dl].T).astype(bf)

        Pm = inp["proj_W"][:, br * 1024:(br + 1) * 1024]
        Om = inp[p + "outW"][:, dl]
        Qm = (Pm.astype(np.float64) @ Om.astype(np.float64)).astype(np.float32)
        m["Q"] = _part_layout(np.ascontiguousarray(Qm.T)).astype(bf)

        xTnat = inp["x"][b].T
        xsl = np.zeros((128, 4, L), np.float32)
        bsl = np.zeros((128, 4), np.float32)
        for j in range(4):
            r0 = h * 512 + j * 128
            xsl[:, j, :] = xTnat[r0:r0 + 128]
            bsl[:, j] = inp["proj_b"][r0:r0 + 128]
        m["xsl"] = xsl
        m["bsl"] = bsl
        m["ident"] = np.eye(128, dtype=np.float32).astype(ml_dtypes.bfloat16)
        in_maps.append(m)
    return in_maps


def assemble_output(results):
    out = np.zeros((B, L, D), np.float32)
    for b in range(B):
        for h in range(2):
            c = b * 2 + h            # fwd cores hold the canonical result
            sl = results[c]["out_sl"]              # [128, 4, L]
            for j in range(4):
                r0 = h * 512 + j * 128
                out[b, :, r0:r0 + 128] = sl[:, j, :].T
    return out


def kernel(**inputs):
    if "prog" not in _CACHED:
        _CACHED["prog"] = build_program()
    nc = _CACHED["prog"]
    in_maps = make_core_inputs(inputs)
    r = run_bass_kernel_spmd(nc, in_maps, list(range(NC)))
    return assemble_output(r.results)
